# revision 24
# baseline (speedup 1.0000x reference)
"""Trainium2 Bass kernel for nn_BertOutput (binary-quantized BERT output layer).

Computation (see reference):
    w_scale = mean(|W|, axis=1)                  # [H, 1]
    W_q     = w_scale * sign(W)                  # [H, I]
    x_q     = clip * sign(x / clip)              # [B, S, I]
    h       = x_q @ W_q.T + b                    # [B, S, H]
    out     = LayerNorm(h + input_tensor) * gamma + beta

Structural facts exploited:
  * The matmul operands are exactly +-1: representable exactly in fp8e4m3,
    and the K=4096 accumulation of +-1 terms is exact in fp32 PSUM.  The
    per-output-channel scale (|clip| * mean|W|) is applied after the matmul.
  * fp8 enables MatmulPerfMode.DoubleRow: one instruction contracts TWO
    128-deep k-subtiles (157 TF/s peak), halving tensor-engine time vs bf16.
  * Sign bits survive the fp32->bf16 cast done during the DMA load.  x signs
    are packed PAIRWISE into u16 words -- fp8 sign of x[t, c] in the low
    byte, fp8 sign of x[t, 2048 + c] in the high byte -- with 3 contiguous
    DVE bitwise ops.  A single 2-byte DMA-xbar transpose then moves BOTH
    fp8 k-planes at once (halving x transpose traffic), and the resulting
    byte-interleaved layout is exactly what LDWEIGHTS perf mode
    DoubleRowSwInterleave consumes natively.  SwInterleave loads the first
    element to the largest PE column (reversing token order), which is
    cancelled by assigning tokens to SBUF partitions in reverse order when
    the shard is prepared on the host (a pure row permutation).
  * W signs are packed the same way; the moving operand reads the
    interleaved bytes through a strided AP (i stride 1, h stride 2), which
    the matmul accepts (HW-verified).
  * |W| row-sums ride on the scalar engine via activation(Abs, accum_out=),
    keeping the DVE free for packing + the LayerNorm epilogue.

Sharding: plain data-parallel over tokens -- 8192 tokens -> 1024 per core.
Each core computes a full LayerNorm over hidden=1024, so no collectives.
"""

import sys

sys.path.insert(0, "/opt/trn_rl_repo")

import numpy as np

import concourse.bass as bass  # noqa: F401  (import side effects / registry)
import concourse.tile as tile
from concourse import bacc, bass_utils, mybir

F32 = mybir.dt.float32
BF16 = mybir.dt.bfloat16
FP8 = mybir.dt.float8e4
U16 = mybir.dt.uint16

HIDDEN = 1024
INTER = 4096
TOKENS = 8192
N_CORES = 8
TPC = TOKENS // N_CORES          # tokens per core = 1024
M_TILES = TPC // 128             # 8 token tiles per core
H_TILES = HIDDEN // 128          # 8 row tiles of W
NBLK = INTER // 256              # 16 double-k-blocks (DoubleRow: 256 k each)
HALF = INTER // 2                # 2048: pack pairs (k, k + HALF)
EPS = 1e-12

TRACE = False                    # set True from test harness to profile
TRACE_ALL_CORES = False

_cache: dict = {}


def _install_ntff_hook():
    """The agent image's antenv package lacks axon_hooks, which silently
    disables NTFF profiling under axon.  Recreate it and wire the ctypes
    hook from trn_agent_boot (profiling/tooling only; the compute path
    does not depend on this)."""
    import types

    import antenv
    if getattr(antenv, "axon_hooks", None) is not None:
        return
    from trn_agent_boot.trn_boot import _ntff_profile_via_ctypes
    mod = types.ModuleType("antenv.axon_hooks")
    mod._hook = _ntff_profile_via_ctypes("/opt/axon/libaxon_pjrt.so")
    mod.get_axon_ntff_profile_hook = lambda: mod._hook

    def _set(h):
        mod._hook = h
    mod.set_axon_ntff_profile_hook = _set
    sys.modules["antenv.axon_hooks"] = mod
    antenv.axon_hooks = mod


def _prepare_x(x_shard: np.ndarray) -> np.ndarray:
    """Sharding-time row permutation: within each 128-token tile, tokens are
    assigned to SBUF partitions in REVERSE order, cancelling SwInterleave's
    first-element-to-largest-column reversal so psum rows come out natural."""
    t = x_shard.reshape(M_TILES, 128, INTER)
    return np.ascontiguousarray(t[:, ::-1, :]).reshape(TPC, INTER)


def _emit_pack(nc, pool, src, dst, tag):
    """Pack sign bits of a bf16 [128, 4096] tile into u16 fp8-sign pairs.

    dst u16 [128, 2048]: word c = lo byte fp8sign(src[:, c]),
                                  hi byte fp8sign(src[:, HALF + c]).
    fp8e4m3 +-1.0 is 0x38 / 0xB8, so:
        lo = (bf16_bits >> 8) & 0x0080  OR'd with  0x0038-from-tsB's 0x3838
        hi = (bf16_bits & 0x8000) | 0x3800
    """
    AT = mybir.AluOpType
    a = src[:, 0:HALF].bitcast(U16)
    b = src[:, HALF:INTER].bitcast(U16)
    tsA = pool.tile([128, HALF], U16, tag=f"{tag}A")
    nc.vector.tensor_scalar(out=tsA[:], in0=a, scalar1=8, scalar2=0x0080,
                            op0=AT.logical_shift_right, op1=AT.bitwise_and)
    tsB = pool.tile([128, HALF], U16, tag=f"{tag}B")
    nc.vector.tensor_scalar(out=tsB[:], in0=b, scalar1=0x8000, scalar2=0x3838,
                            op0=AT.bitwise_and, op1=AT.bitwise_or)
    nc.vector.tensor_tensor(out=dst[:], in0=tsA[:], in1=tsB[:],
                            op=AT.bitwise_or)


def _emit_program(nc, x_ap, res_ap, w_ap, y_ap, b_ap, g_ap, be_ap,
                  scale_mul: float, use_b: bool, use_gamma: bool,
                  use_beta: bool):
    """Emit the per-core Tile program given DRAM APs."""
    AT = mybir.AluOpType
    AF = mybir.ActivationFunctionType
    DRSI = mybir.MatmulPerfMode.DoubleRowSwInterleave

    with tile.TileContext(nc) as tc:
        with (
            tc.tile_pool(name="wt", bufs=1) as wt_pool,
            tc.tile_pool(name="wstage", bufs=3) as wstage_pool,
            tc.tile_pool(name="wpk", bufs=2) as wpk_pool,
            tc.tile_pool(name="const", bufs=1) as const_pool,
            tc.tile_pool(name="xio", bufs=3) as xio_pool,
            tc.tile_pool(name="xpk", bufs=2) as xpk_pool,
            tc.tile_pool(name="xt", bufs=3) as xt_pool,
            tc.tile_pool(name="res", bufs=4) as res_pool,
            tc.tile_pool(name="epi", bufs=2) as epi_pool,
            tc.tile_pool(name="stats", bufs=2) as stats_pool,
            tc.tile_pool(name="psum", bufs=4, space="PSUM") as psum_pool,
            tc.tile_pool(name="dram", bufs=1, space="DRAM") as dram_pool,
        ):
            epsT = const_pool.tile([128, 1], F32, tag="epsT")
            nc.vector.memset(epsT[:], float(EPS))

            # ---------------- DMA dispatch: W loads first ----------------
            # All big HBM loads go through the gpsimd (SWDGE) ring so they
            # drain in exactly this order: W0..W7, then x/res interleaved.
            wlds = []
            for ht in range(H_TILES):
                wld = wstage_pool.tile([128, INTER], BF16, tag="wld")
                nc.gpsimd.dma_start(wld[:], w_ap[ht * 128:(ht + 1) * 128, :])
                wlds.append(wld)

            xins, inps = [], []
            for m in range(M_TILES):
                xin = xio_pool.tile([128, INTER], BF16, tag="xin")
                nc.gpsimd.dma_start(xin[:], x_ap[m * 128:(m + 1) * 128, :])
                xins.append(xin)
                inp = res_pool.tile([128, HIDDEN], F32, tag="inp")
                nc.gpsimd.dma_start(inp[:], res_ap[m * 128:(m + 1) * 128, :])
                inps.append(inp)

            # ---------------- W preparation ----------------
            # wT8[n] fp8 [128, 32, 512]: (p, kb, h') = fp8 sign W[n*512+h',
            # kb*128+p], h contiguous so the moving operand streams at full
            # rate.  DoubleRow rhs block b = [:, b::16, :] (pairs kb b and
            # b+16, matching the x pack pairing (c, 2048+c)).
            wT8 = [wt_pool.tile([128, 2 * NBLK, 512], FP8, tag=f"wT8{n}",
                                name=f"wT8{n}") for n in range(2)]
            wsum = const_pool.tile([128, H_TILES], F32, tag="wsum")

            for ht in range(H_TILES):
                wld = wlds[ht]
                # |W| row-sum (fp32 accumulation of exact bf16 |w|)
                nc.vector.tensor_reduce(wsum[:, ht:ht + 1], wld[:],
                                        axis=mybir.AxisListType.X, op=AT.add,
                                        apply_absolute_value=True)
                # sign via bit trick: (v & 0x8000) | 0x3F80 -> +-1.0 bf16
                wsgn = wpk_pool.tile([128, INTER], BF16, tag="wsgn")
                nc.vector.tensor_scalar(
                    out=wsgn[:].bitcast(U16), in0=wld[:].bitcast(U16),
                    scalar1=0x8000, scalar2=0x3F80,
                    op0=AT.bitwise_and, op1=AT.bitwise_or)
                wTb = wpk_pool.tile([128, 2 * NBLK, 128], BF16, tag="wTb")
                nc.sync.dma_start(out=wTb[:], in_=wsgn[:], transpose=True)
                n, hl = ht // 4, ht % 4
                # bf16 -> fp8 convert on the ACT engine (strided dst)
                nc.scalar.copy(wT8[n][:, :, hl * 128:(hl + 1) * 128], wTb[:])

            # per-channel scale: scale[h] = |clip|/INTER * sum_i |W[h, i]|
            # DRAM round-trip converts partition-major -> free-major, then
            # a broadcast load fans it out to all 128 partitions.
            scale8 = const_pool.tile([128, H_TILES], F32, tag="scale8")
            nc.vector.tensor_scalar(out=scale8[:], in0=wsum[:],
                                    scalar1=float(scale_mul), scalar2=None,
                                    op0=AT.mult)
            scratch = dram_pool.tile([HIDDEN], F32)
            # h = j*128 + p  ->  dram[j*128+p] = scale8[p, j]
            nc.sync.dma_start(
                out=scratch[:].rearrange("(j p) -> p j", p=128), in_=scale8[:])
            scaleF = const_pool.tile([128, HIDDEN], F32, tag="scaleF")
            nc.sync.dma_start(
                scaleF[:],
                scratch[:].rearrange("(a n) -> a n", a=1).broadcast_to([128, HIDDEN]))

            bB = gB = beB = None
            if use_b:
                bB = const_pool.tile([128, HIDDEN], F32, tag="bB")
                nc.sync.dma_start(
                    bB[:],
                    b_ap.rearrange("(a n) -> a n", a=1).broadcast_to([128, HIDDEN]))
            if use_gamma:
                gB = const_pool.tile([128, HIDDEN], F32, tag="gB")
                nc.sync.dma_start(
                    gB[:],
                    g_ap.rearrange("(a n) -> a n", a=1).broadcast_to([128, HIDDEN]))
            if use_beta:
                beB = const_pool.tile([128, HIDDEN], F32, tag="beB")
                nc.sync.dma_start(
                    beB[:],
                    be_ap.rearrange("(a n) -> a n", a=1).broadcast_to([128, HIDDEN]))

            # ---------------- main loop over token tiles ----------------
            for m in range(M_TILES):
                xin = xins[m]
                xpackU = xpk_pool.tile([128, HALF], U16, tag="xpackU")
                _emit_pack(nc, xpk_pool, xin, xpackU, "xts")
                xTp = xt_pool.tile([128, NBLK, 128], U16, tag="xTp")
                nc.scalar.dma_start(out=xTp[:], in_=xpackU[:], transpose=True)

                psum = psum_pool.tile([128, HIDDEN], F32, tag="psum",
                                      name="ps")
                for b in range(NBLK):
                    # forward interleaved byte-pairs; SwInterleave's column
                    # reversal is cancelled by the host-side row reversal
                    lhsT = xTp[:, b, :].bitcast(FP8)
                    for n in range(2):
                        nc.tensor.matmul(
                            psum[:, n * 512:(n + 1) * 512],
                            lhsT=lhsT,
                            rhs=wT8[n][:, b::NBLK, :],
                            start=(b == 0), stop=(b == NBLK - 1),
                            perf_mode=DRSI)

                # epilogue: r = psum * scaleF + inp (+ bB), then LayerNorm
                inp = inps[m]
                t = epi_pool.tile([128, HIDDEN], F32, tag="t")
                nc.vector.tensor_mul(t[:], psum[:], scaleF[:])
                r = epi_pool.tile([128, HIDDEN], F32, tag="r")
                nc.vector.tensor_add(r[:], t[:], inp[:])
                if use_b:
                    r2 = epi_pool.tile([128, HIDDEN], F32, tag="r2")
                    nc.vector.tensor_add(r2[:], r[:], bB[:])
                    r = r2

                bn6 = stats_pool.tile([128, 2, 6], F32, tag="bn6")
                nc.vector.bn_stats(bn6[:, 0, :], r[:, 0:512])
                nc.vector.bn_stats(bn6[:, 1, :], r[:, 512:1024])
                mv = stats_pool.tile([128, 2], F32, tag="mv")
                nc.vector.bn_aggr(mv[:], bn6[:])
                sd = stats_pool.tile([128, 1], F32, tag="sd")
                nc.scalar.activation(sd[:], mv[:, 1:2], AF.Sqrt,
                                     bias=epsT[:, 0:1])
                rstd = stats_pool.tile([128, 1], F32, tag="rstd")
                nc.vector.reciprocal(rstd[:], sd[:])
                nm = stats_pool.tile([128, 1], F32, tag="nm")
                nc.vector.tensor_scalar(out=nm[:], in0=mv[:, 0:1],
                                        scalar1=rstd[:, 0:1], scalar2=-1.0,
                                        op0=AT.mult, op1=AT.mult)
                y = epi_pool.tile([128, HIDDEN], F32, tag="y")
                nc.scalar.activation(y[:], r[:], AF.Identity,
                                     bias=nm[:, 0:1], scale=rstd[:, 0:1])
                if use_gamma:
                    y2 = epi_pool.tile([128, HIDDEN], F32, tag="y2")
                    nc.vector.tensor_mul(y2[:], y[:], gB[:])
                    y = y2
                if use_beta:
                    y3 = epi_pool.tile([128, HIDDEN], F32, tag="y3")
                    nc.vector.tensor_add(y3[:], y[:], beB[:])
                    y = y3

                nc.sync.dma_start(y_ap[m * 128:(m + 1) * 128, :], y[:])


def _build(scale_mul: float, use_b: bool, use_gamma: bool, use_beta: bool):
    """Build the SPMD program (identical on all 8 cores).

    scale_mul = |clip_val| / INTER, folded into the per-channel scale.
    """
    nc = bacc.Bacc("TRN2", target_bir_lowering=False, debug=False,
                   num_devices=N_CORES)

    x_ap = nc.dram_tensor("x", [TPC, INTER], F32, kind="ExternalInput").ap()
    res_ap = nc.dram_tensor("res", [TPC, HIDDEN], F32, kind="ExternalInput").ap()
    w_ap = nc.dram_tensor("w", [HIDDEN, INTER], F32, kind="ExternalInput").ap()
    b_ap = g_ap = be_ap = None
    if use_b:
        b_ap = nc.dram_tensor("bvec", [HIDDEN], F32, kind="ExternalInput").ap()
    if use_gamma:
        g_ap = nc.dram_tensor("gvec", [HIDDEN], F32, kind="ExternalInput").ap()
    if use_beta:
        be_ap = nc.dram_tensor("bevec", [HIDDEN], F32, kind="ExternalInput").ap()
    y_ap = nc.dram_tensor("y", [TPC, HIDDEN], F32, kind="ExternalOutput").ap()

    _emit_program(nc, x_ap, res_ap, w_ap, y_ap, b_ap, g_ap, be_ap,
                  scale_mul, use_b, use_gamma, use_beta)
    nc.compile()
    return nc


_last_results = None


def kernel(hidden_states, input_tensor, W, b, clip_val, gamma, beta):
    global _last_results
    hidden_states = np.asarray(hidden_states)
    input_tensor = np.asarray(input_tensor)
    W = np.asarray(W, dtype=np.float32)
    b = np.asarray(b, dtype=np.float32)
    gamma = np.asarray(gamma, dtype=np.float32)
    beta = np.asarray(beta, dtype=np.float32)
    clip = float(np.asarray(clip_val))

    use_b = bool(np.any(b != 0.0))
    use_gamma = bool(np.any(gamma != 1.0))
    use_beta = bool(np.any(beta != 0.0))
    scale_mul = abs(clip) / INTER

    key = (scale_mul, use_b, use_gamma, use_beta)
    if key not in _cache:
        _cache[key] = _build(scale_mul, use_b, use_gamma, use_beta)
    nc = _cache[key]

    hs = np.ascontiguousarray(
        hidden_states.reshape(TOKENS, INTER).astype(np.float32, copy=False))
    rs = np.ascontiguousarray(
        input_tensor.reshape(TOKENS, HIDDEN).astype(np.float32, copy=False))
    Wc = np.ascontiguousarray(W)

    in_maps = []
    for c in range(N_CORES):
        m = {
            "x": _prepare_x(hs[c * TPC:(c + 1) * TPC]),
            "res": np.ascontiguousarray(rs[c * TPC:(c + 1) * TPC]),
            "w": Wc,
        }
        if use_b:
            m["bvec"] = b
        if use_gamma:
            m["gvec"] = gamma
        if use_beta:
            m["bevec"] = beta
        in_maps.append(m)

    kwargs = {}
    if TRACE:
        _install_ntff_hook()
        kwargs["trace"] = True
        if TRACE_ALL_CORES:
            kwargs["trace_cores"] = list(range(N_CORES))
    res = bass_utils.run_bass_kernel_spmd(
        nc, in_maps, core_ids=list(range(N_CORES)), **kwargs)
    _last_results = res

    y = np.concatenate([res.results[c]["y"] for c in range(N_CORES)], axis=0)
    return y.reshape(hidden_states.shape[:-1] + (HIDDEN,)).astype(np.float32)


# revision 27
# speedup vs baseline: 1.4832x; 1.4832x over previous
"""Trainium2 Bass kernel for nn_BertOutput (binary-quantized BERT output layer).

Computation (see reference):
    w_scale = mean(|W|, axis=1)                  # [H, 1]
    W_q     = w_scale * sign(W)                  # [H, I]
    x_q     = clip * sign(x / clip)              # [B, S, I]
    h       = x_q @ W_q.T + b                    # [B, S, H]
    out     = LayerNorm(h + input_tensor) * gamma + beta

Structural facts exploited:
  * The matmul operands are exactly +-1: representable exactly in fp8e4m3,
    and the K=4096 accumulation of +-1 terms is exact in fp32 PSUM.  The
    per-output-channel scale (|clip| * mean|W|) is applied after the matmul.
  * fp8 enables MatmulPerfMode.DoubleRow: one instruction contracts TWO
    128-deep k-subtiles (157 TF/s peak), halving tensor-engine time vs bf16.
  * Sign bits survive the fp32->bf16 cast done during the DMA load.  x signs
    are packed PAIRWISE into u16 words -- fp8 sign of x[t, c] in the low
    byte, fp8 sign of x[t, 2048 + c] in the high byte -- with 3 contiguous
    DVE bitwise ops.  One 2-byte transpose then moves BOTH fp8 k-planes at
    once, and the byte-interleaved result is exactly what LDWEIGHTS perf
    mode DoubleRowSwInterleave consumes natively.  SwInterleave loads the first
    element to the largest PE column (reversing token order), which is
    cancelled by assigning tokens to SBUF partitions in reverse order when
    the shard is prepared on the host (a pure row permutation).
  * W is fed TRANSPOSED from the host (a pure layout/sharding choice), so
    it lands k-major and needs no on-device transpose; fp8 signs are taken
    directly by the scalar engine.  The moving operand [K, 2, N] slices are
    h-contiguous for full-rate streaming.
  * DMA-xbar transposes turned out to lock ALL DMA engines for their whole
    duration (they cannot overlap the HBM loads), so the x tile transposes
    run on the PE array instead (is_transpose matmul; the packed u16 words
    are bf16 normals, so a bf16 PE transpose is bit-exact, HW-verified).
  * The per-channel scale numerator sum_k |W[h,k]| is a cross-partition
    reduction in the W^T layout, computed as ones.T @ |W^T| on the PE.

Sharding: plain data-parallel over tokens -- 8192 tokens -> 1024 per core.
Each core computes a full LayerNorm over hidden=1024, so no collectives.
"""

import sys

sys.path.insert(0, "/opt/trn_rl_repo")

import numpy as np

import concourse.bass as bass  # noqa: F401  (import side effects / registry)
import concourse.tile as tile
from concourse import bacc, bass_utils, mybir

F32 = mybir.dt.float32
BF16 = mybir.dt.bfloat16
FP8 = mybir.dt.float8e4
U16 = mybir.dt.uint16

HIDDEN = 1024
INTER = 4096
TOKENS = 8192
N_CORES = 8
TPC = TOKENS // N_CORES          # tokens per core = 1024
M_TILES = TPC // 128             # 8 token tiles per core
H_TILES = HIDDEN // 128          # 8 row tiles of W
NBLK = INTER // 256              # 16 double-k-blocks (DoubleRow: 256 k each)
HALF = INTER // 2                # 2048: pack pairs (k, k + HALF)
EPS = 1e-12

TRACE = False                    # set True from test harness to profile
TRACE_ALL_CORES = False

_cache: dict = {}


def _install_ntff_hook():
    """The agent image's antenv package lacks axon_hooks, which silently
    disables NTFF profiling under axon.  Recreate it and wire the ctypes
    hook from trn_agent_boot (profiling/tooling only; the compute path
    does not depend on this)."""
    import types

    import antenv
    if getattr(antenv, "axon_hooks", None) is not None:
        return
    from trn_agent_boot.trn_boot import _ntff_profile_via_ctypes
    mod = types.ModuleType("antenv.axon_hooks")
    mod._hook = _ntff_profile_via_ctypes("/opt/axon/libaxon_pjrt.so")
    mod.get_axon_ntff_profile_hook = lambda: mod._hook

    def _set(h):
        mod._hook = h
    mod.set_axon_ntff_profile_hook = _set
    sys.modules["antenv.axon_hooks"] = mod
    antenv.axon_hooks = mod


def _prepare_x(x_shard: np.ndarray) -> np.ndarray:
    """Sharding-time row permutation: within each 128-token tile, tokens are
    assigned to SBUF partitions in REVERSE order, cancelling SwInterleave's
    first-element-to-largest-column reversal so psum rows come out natural."""
    t = x_shard.reshape(M_TILES, 128, INTER)
    return np.ascontiguousarray(t[:, ::-1, :]).reshape(TPC, INTER)


def _emit_pack(nc, pool, src, dst, tag):
    """Pack sign bits of a bf16 [128, 4096] tile into u16 fp8-sign pairs.

    dst u16 [128, 2048]: word c = lo byte fp8sign(src[:, c]),
                                  hi byte fp8sign(src[:, HALF + c]).
    fp8e4m3 +-1.0 is 0x38 / 0xB8, so:
        lo = (bf16_bits >> 8) & 0x0080  OR'd with  0x0038-from-tsB's 0x3838
        hi = (bf16_bits & 0x8000) | 0x3800
    """
    AT = mybir.AluOpType
    a = src[:, 0:HALF].bitcast(U16)
    b = src[:, HALF:INTER].bitcast(U16)
    tsA = pool.tile([128, HALF], U16, tag=f"{tag}A")
    nc.vector.tensor_scalar(out=tsA[:], in0=a, scalar1=8, scalar2=0x0080,
                            op0=AT.logical_shift_right, op1=AT.bitwise_and)
    tsB = pool.tile([128, HALF], U16, tag=f"{tag}B")
    nc.vector.tensor_scalar(out=tsB[:], in0=b, scalar1=0x8000, scalar2=0x3838,
                            op0=AT.bitwise_and, op1=AT.bitwise_or)
    nc.vector.tensor_tensor(out=dst[:], in0=tsA[:], in1=tsB[:],
                            op=AT.bitwise_or)


def _emit_program(nc, x_ap, res_ap, wt_ap, y_ap, b_ap, g_ap, be_ap,
                  scale_mul: float, use_b: bool, use_gamma: bool,
                  use_beta: bool):
    """Emit the per-core Tile program given DRAM APs.

    wt_ap is W TRANSPOSED ([INTER, HIDDEN]) -- a host-side layout choice so
    the weight lands k-major and needs no on-device transpose.
    """
    AT = mybir.AluOpType
    AF = mybir.ActivationFunctionType
    DRSI = mybir.MatmulPerfMode.DoubleRowSwInterleave
    K_TILES = INTER // 128            # 32 k-tiles of W^T
    WG = 4                            # k-tiles per W load
    from concourse.masks import make_identity

    with tile.TileContext(nc) as tc:
        with (
            tc.tile_pool(name="wt", bufs=1) as wt_pool,
            tc.tile_pool(name="wstage", bufs=3) as wstage_pool,
            tc.tile_pool(name="wpk", bufs=2) as wpk_pool,
            tc.tile_pool(name="const", bufs=1) as const_pool,
            tc.tile_pool(name="xio", bufs=3) as xio_pool,
            tc.tile_pool(name="xpk", bufs=2) as xpk_pool,
            tc.tile_pool(name="xt", bufs=3) as xt_pool,
            tc.tile_pool(name="res", bufs=4) as res_pool,
            tc.tile_pool(name="epi", bufs=2) as epi_pool,
            tc.tile_pool(name="stats", bufs=2) as stats_pool,
            tc.tile_pool(name="psum", bufs=2, space="PSUM") as psum_pool,
            tc.tile_pool(name="pst", bufs=2, space="PSUM") as pst_pool,
            tc.tile_pool(name="wsps", bufs=1, space="PSUM") as wsps_pool,
            tc.tile_pool(name="dram", bufs=1, space="DRAM") as dram_pool,
        ):
            epsT = const_pool.tile([128, 1], F32, tag="epsT")
            nc.vector.memset(epsT[:], float(EPS))
            ones1 = const_pool.tile([128, 1], BF16, tag="ones1")
            nc.vector.memset(ones1[:], 1.0)
            ident = const_pool.tile([128, 128], BF16, tag="ident")
            make_identity(nc, ident[:])

            # ---------------- DMA dispatch: W loads first ----------------
            # All big HBM loads go through the gpsimd (SWDGE) ring so they
            # drain in exactly this order: W^T groups, then x/res interleaved.
            wlds = []
            for g in range(K_TILES // WG):
                wld = wstage_pool.tile([128, WG, HIDDEN], BF16, tag="wld")
                nc.gpsimd.dma_start(
                    wld[:],
                    wt_ap[g * WG * 128:(g + 1) * WG * 128, :].rearrange(
                        "(c p) h -> p c h", p=128))
                wlds.append(wld)

            xins, inps = [], []
            for m in range(M_TILES):
                xin = xio_pool.tile([128, INTER], BF16, tag="xin")
                nc.gpsimd.dma_start(xin[:], x_ap[m * 128:(m + 1) * 128, :])
                xins.append(xin)
                inp = res_pool.tile([128, HIDDEN], F32, tag="inp")
                nc.gpsimd.dma_start(inp[:], res_ap[m * 128:(m + 1) * 128, :])
                inps.append(inp)

            # ---------------- W preparation (no transposes) ----------------
            # wT8 fp8 [128, 32, 1024]: (p, kt, h) = fp8 sign W[h, kt*128+p].
            # DoubleRow rhs block b, half n = [:, b::16, n*512:(n+1)*512]
            # (k-pair (b, b+16) matches the x pack pairing (c, 2048+c)).
            wT8 = wt_pool.tile([128, K_TILES, HIDDEN], FP8, tag="wT8",
                               name="wT8")
            wsps = wsps_pool.tile([1, HIDDEN], F32, tag="wsps", name="wsps")

            for kt in range(K_TILES):
                g, c = kt // WG, kt % WG
                wld = wlds[g]
                # fp8 sign directly on the scalar engine
                nc.scalar.sign(wT8[:, kt, :], wld[:, c, :])
                # |w| via sign-bit mask, then ones.T @ |w| accumulates the
                # per-channel scale numerator on the PE
                wabs = wpk_pool.tile([128, HIDDEN], BF16, tag="wabs")
                nc.vector.tensor_scalar(
                    out=wabs[:].bitcast(U16), in0=wld[:, c, :].bitcast(U16),
                    scalar1=0x7FFF, scalar2=None, op0=AT.bitwise_and)
                for n in range(2):
                    nc.tensor.matmul(wsps[:, n * 512:(n + 1) * 512],
                                     lhsT=ones1[:],
                                     rhs=wabs[:, n * 512:(n + 1) * 512],
                                     start=(kt == 0),
                                     stop=(kt == K_TILES - 1))

            # scale row -> DRAM -> broadcast to all partitions
            srow = const_pool.tile([1, HIDDEN], F32, tag="srow")
            nc.scalar.activation(srow[:], wsps[:], AF.Copy,
                                 scale=float(scale_mul))
            scratch = dram_pool.tile([HIDDEN], F32)
            nc.sync.dma_start(
                out=scratch[:].rearrange("(a n) -> a n", a=1), in_=srow[:])
            scaleF = const_pool.tile([128, HIDDEN], F32, tag="scaleF")
            nc.sync.dma_start(
                scaleF[:],
                scratch[:].rearrange("(a n) -> a n", a=1).broadcast_to([128, HIDDEN]))

            bB = gB = beB = None
            if use_b:
                bB = const_pool.tile([128, HIDDEN], F32, tag="bB")
                nc.sync.dma_start(
                    bB[:],
                    b_ap.rearrange("(a n) -> a n", a=1).broadcast_to([128, HIDDEN]))
            if use_gamma:
                gB = const_pool.tile([128, HIDDEN], F32, tag="gB")
                nc.sync.dma_start(
                    gB[:],
                    g_ap.rearrange("(a n) -> a n", a=1).broadcast_to([128, HIDDEN]))
            if use_beta:
                beB = const_pool.tile([128, HIDDEN], F32, tag="beB")
                nc.sync.dma_start(
                    beB[:],
                    be_ap.rearrange("(a n) -> a n", a=1).broadcast_to([128, HIDDEN]))

            # ---------------- main loop over token tiles ----------------
            for m in range(M_TILES):
                xin = xins[m]
                xpackU = xpk_pool.tile([128, HALF], U16, tag="xpackU")
                _emit_pack(nc, xpk_pool, xin, xpackU, "xts")
                # transpose the 16 packed blocks on the PE (bit-exact for
                # the 4 sign-pair bf16 normals), staging through PSUM
                xTp = xt_pool.tile([128, NBLK, 128], U16, tag="xTp")
                for grp in range(2):
                    pst = pst_pool.tile([128, 8, 128], BF16, tag="pst")
                    for j in range(8):
                        blk = grp * 8 + j
                        nc.tensor.transpose(
                            pst[:, j, :],
                            xpackU[:, blk * 128:(blk + 1) * 128].bitcast(BF16),
                            ident[:])
                    nc.vector.tensor_copy(
                        xTp[:, grp * 8:(grp + 1) * 8, :].bitcast(BF16),
                        pst[:])

                psum = psum_pool.tile([128, HIDDEN], F32, tag="psum",
                                      name="ps")
                for b in range(NBLK):
                    # forward interleaved byte-pairs; SwInterleave's column
                    # reversal is cancelled by the host-side row reversal
                    lhsT = xTp[:, b, :].bitcast(FP8)
                    for n in range(2):
                        nc.tensor.matmul(
                            psum[:, n * 512:(n + 1) * 512],
                            lhsT=lhsT,
                            rhs=wT8[:, b::NBLK, n * 512:(n + 1) * 512],
                            start=(b == 0), stop=(b == NBLK - 1),
                            perf_mode=DRSI)

                # epilogue: r = psum * scaleF + inp (+ bB), then LayerNorm
                inp = inps[m]
                t = epi_pool.tile([128, HIDDEN], F32, tag="t")
                nc.vector.tensor_mul(t[:], psum[:], scaleF[:])
                r = epi_pool.tile([128, HIDDEN], F32, tag="r")
                nc.vector.tensor_add(r[:], t[:], inp[:])
                if use_b:
                    r2 = epi_pool.tile([128, HIDDEN], F32, tag="r2")
                    nc.vector.tensor_add(r2[:], r[:], bB[:])
                    r = r2

                bn6 = stats_pool.tile([128, 2, 6], F32, tag="bn6")
                nc.vector.bn_stats(bn6[:, 0, :], r[:, 0:512])
                nc.vector.bn_stats(bn6[:, 1, :], r[:, 512:1024])
                mv = stats_pool.tile([128, 2], F32, tag="mv")
                nc.vector.bn_aggr(mv[:], bn6[:])
                sd = stats_pool.tile([128, 1], F32, tag="sd")
                nc.scalar.activation(sd[:], mv[:, 1:2], AF.Sqrt,
                                     bias=epsT[:, 0:1])
                rstd = stats_pool.tile([128, 1], F32, tag="rstd")
                nc.vector.reciprocal(rstd[:], sd[:])
                nm = stats_pool.tile([128, 1], F32, tag="nm")
                nc.vector.tensor_scalar(out=nm[:], in0=mv[:, 0:1],
                                        scalar1=rstd[:, 0:1], scalar2=-1.0,
                                        op0=AT.mult, op1=AT.mult)
                y = epi_pool.tile([128, HIDDEN], F32, tag="y")
                nc.scalar.activation(y[:], r[:], AF.Identity,
                                     bias=nm[:, 0:1], scale=rstd[:, 0:1])
                if use_gamma:
                    y2 = epi_pool.tile([128, HIDDEN], F32, tag="y2")
                    nc.vector.tensor_mul(y2[:], y[:], gB[:])
                    y = y2
                if use_beta:
                    y3 = epi_pool.tile([128, HIDDEN], F32, tag="y3")
                    nc.vector.tensor_add(y3[:], y[:], beB[:])
                    y = y3

                nc.sync.dma_start(y_ap[m * 128:(m + 1) * 128, :], y[:])


def _build(scale_mul: float, use_b: bool, use_gamma: bool, use_beta: bool):
    """Build the SPMD program (identical on all 8 cores).

    scale_mul = |clip_val| / INTER, folded into the per-channel scale.
    """
    nc = bacc.Bacc("TRN2", target_bir_lowering=False, debug=False,
                   num_devices=N_CORES)

    x_ap = nc.dram_tensor("x", [TPC, INTER], F32, kind="ExternalInput").ap()
    res_ap = nc.dram_tensor("res", [TPC, HIDDEN], F32, kind="ExternalInput").ap()
    wt_ap = nc.dram_tensor("wt", [INTER, HIDDEN], F32, kind="ExternalInput").ap()
    b_ap = g_ap = be_ap = None
    if use_b:
        b_ap = nc.dram_tensor("bvec", [HIDDEN], F32, kind="ExternalInput").ap()
    if use_gamma:
        g_ap = nc.dram_tensor("gvec", [HIDDEN], F32, kind="ExternalInput").ap()
    if use_beta:
        be_ap = nc.dram_tensor("bevec", [HIDDEN], F32, kind="ExternalInput").ap()
    y_ap = nc.dram_tensor("y", [TPC, HIDDEN], F32, kind="ExternalOutput").ap()

    _emit_program(nc, x_ap, res_ap, wt_ap, y_ap, b_ap, g_ap, be_ap,
                  scale_mul, use_b, use_gamma, use_beta)
    nc.compile()
    return nc


_last_results = None


def kernel(hidden_states, input_tensor, W, b, clip_val, gamma, beta):
    global _last_results
    hidden_states = np.asarray(hidden_states)
    input_tensor = np.asarray(input_tensor)
    W = np.asarray(W, dtype=np.float32)
    b = np.asarray(b, dtype=np.float32)
    gamma = np.asarray(gamma, dtype=np.float32)
    beta = np.asarray(beta, dtype=np.float32)
    clip = float(np.asarray(clip_val))

    use_b = bool(np.any(b != 0.0))
    use_gamma = bool(np.any(gamma != 1.0))
    use_beta = bool(np.any(beta != 0.0))
    scale_mul = abs(clip) / INTER

    key = (scale_mul, use_b, use_gamma, use_beta)
    if key not in _cache:
        _cache[key] = _build(scale_mul, use_b, use_gamma, use_beta)
    nc = _cache[key]

    hs = np.ascontiguousarray(
        hidden_states.reshape(TOKENS, INTER).astype(np.float32, copy=False))
    rs = np.ascontiguousarray(
        input_tensor.reshape(TOKENS, HIDDEN).astype(np.float32, copy=False))
    Wc = np.ascontiguousarray(W.T)   # layout choice: weight fed k-major

    in_maps = []
    for c in range(N_CORES):
        m = {
            "x": _prepare_x(hs[c * TPC:(c + 1) * TPC]),
            "res": np.ascontiguousarray(rs[c * TPC:(c + 1) * TPC]),
            "wt": Wc,
        }
        if use_b:
            m["bvec"] = b
        if use_gamma:
            m["gvec"] = gamma
        if use_beta:
            m["bevec"] = beta
        in_maps.append(m)

    kwargs = {}
    if TRACE:
        _install_ntff_hook()
        kwargs["trace"] = True
        if TRACE_ALL_CORES:
            kwargs["trace_cores"] = list(range(N_CORES))
    res = bass_utils.run_bass_kernel_spmd(
        nc, in_maps, core_ids=list(range(N_CORES)), **kwargs)
    _last_results = res

    y = np.concatenate([res.results[c]["y"] for c in range(N_CORES)], axis=0)
    return y.reshape(hidden_states.shape[:-1] + (HIDDEN,)).astype(np.float32)


# revision 31
# speedup vs baseline: 1.7118x; 1.1542x over previous
"""Trainium2 Bass kernel for nn_BertOutput (binary-quantized BERT output layer).

Computation (see reference):
    w_scale = mean(|W|, axis=1)                  # [H, 1]
    W_q     = w_scale * sign(W)                  # [H, I]
    x_q     = clip * sign(x / clip)              # [B, S, I]
    h       = x_q @ W_q.T + b                    # [B, S, H]
    out     = LayerNorm(h + input_tensor) * gamma + beta

Structural facts exploited:
  * The matmul operands are exactly +-1: representable exactly in fp8e4m3,
    and the K=4096 accumulation of +-1 terms is exact in fp32 PSUM.  The
    per-output-channel scale (|clip| * mean|W|) is applied after the matmul.
  * fp8 enables MatmulPerfMode.DoubleRow: one instruction contracts TWO
    128-deep k-subtiles (157 TF/s peak), halving tensor-engine time vs bf16.
  * Sign bits survive the fp32->bf16 cast done during the DMA load.  x signs
    are packed PAIRWISE into u16 words -- fp8 sign of x[t, c] in the low
    byte, fp8 sign of x[t, 2048 + c] in the high byte -- with 3 contiguous
    DVE bitwise ops.  One 2-byte transpose then moves BOTH fp8 k-planes at
    once, and the byte-interleaved result is exactly what LDWEIGHTS perf
    mode DoubleRowSwInterleave consumes natively.  SwInterleave loads the first
    element to the largest PE column (reversing token order), which is
    cancelled by assigning tokens to SBUF partitions in reverse order when
    the shard is prepared on the host (a pure row permutation).
  * W is fed TRANSPOSED from the host (a pure layout/sharding choice), so
    it lands k-major and needs no on-device transpose; fp8 signs are taken
    directly by the scalar engine.  The moving operand [K, 2, N] slices are
    h-contiguous for full-rate streaming.
  * DMA-xbar transposes turned out to lock ALL DMA engines for their whole
    duration (they cannot overlap the HBM loads), so the x tile transposes
    run on the PE array instead (is_transpose matmul; the packed u16 words
    are bf16 normals, so a bf16 PE transpose is bit-exact, HW-verified).
  * The per-channel scale numerator sum_k |W[h,k]| is a cross-partition
    reduction in the W^T layout, computed as ones.T @ |W^T| on the PE.

Sharding: plain data-parallel over tokens -- 8192 tokens -> 1024 per core.
Each core computes a full LayerNorm over hidden=1024, so no collectives.
"""

import sys

sys.path.insert(0, "/opt/trn_rl_repo")

import numpy as np

import concourse.bass as bass  # noqa: F401  (import side effects / registry)
import concourse.tile as tile
from concourse import bacc, bass_utils, mybir

F32 = mybir.dt.float32
BF16 = mybir.dt.bfloat16
FP8 = mybir.dt.float8e4
U16 = mybir.dt.uint16

HIDDEN = 1024
INTER = 4096
TOKENS = 8192
N_CORES = 8
TPC = TOKENS // N_CORES          # tokens per core = 1024
M_TILES = TPC // 128             # 8 token tiles per core
H_TILES = HIDDEN // 128          # 8 row tiles of W
NBLK = INTER // 256              # 16 double-k-blocks (DoubleRow: 256 k each)
HALF = INTER // 2                # 2048: pack pairs (k, k + HALF)
EPS = 1e-12

TRACE = False                    # set True from test harness to profile
TRACE_ALL_CORES = False

_cache: dict = {}


def _install_ntff_hook():
    """The agent image's antenv package lacks axon_hooks, which silently
    disables NTFF profiling under axon.  Recreate it and wire the ctypes
    hook from trn_agent_boot (profiling/tooling only; the compute path
    does not depend on this)."""
    import types

    import antenv
    if getattr(antenv, "axon_hooks", None) is not None:
        return
    from trn_agent_boot.trn_boot import _ntff_profile_via_ctypes
    mod = types.ModuleType("antenv.axon_hooks")
    mod._hook = _ntff_profile_via_ctypes("/opt/axon/libaxon_pjrt.so")
    mod.get_axon_ntff_profile_hook = lambda: mod._hook

    def _set(h):
        mod._hook = h
    mod.set_axon_ntff_profile_hook = _set
    sys.modules["antenv.axon_hooks"] = mod
    antenv.axon_hooks = mod


def _prepare_x(x_shard: np.ndarray) -> np.ndarray:
    """Sharding-time row permutation: within each 128-token tile, tokens are
    assigned to SBUF partitions in REVERSE order, cancelling SwInterleave's
    first-element-to-largest-column reversal so psum rows come out natural."""
    t = x_shard.reshape(M_TILES, 128, INTER)
    return np.ascontiguousarray(t[:, ::-1, :]).reshape(TPC, INTER)


def _emit_pack(nc, pool, src, dst, tag):
    """Pack sign bits of a bf16 [128, 4096] tile into u16 fp8-sign pairs.

    dst u16 [128, 2048]: word c = lo byte fp8sign(src[:, c]),
                                  hi byte fp8sign(src[:, HALF + c]).
    fp8e4m3 +-1.0 is 0x38 / 0xB8, so:
        lo = (bf16_bits >> 8) & 0x0080  OR'd with  0x0038-from-tsB's 0x3838
        hi = (bf16_bits & 0x8000) | 0x3800
    """
    AT = mybir.AluOpType
    a = src[:, 0:HALF].bitcast(U16)
    b = src[:, HALF:INTER].bitcast(U16)
    tsA = pool.tile([128, HALF], U16, tag=f"{tag}A")
    nc.vector.tensor_scalar(out=tsA[:], in0=a, scalar1=8, scalar2=0x0080,
                            op0=AT.logical_shift_right, op1=AT.bitwise_and)
    tsB = pool.tile([128, HALF], U16, tag=f"{tag}B")
    nc.vector.tensor_scalar(out=tsB[:], in0=b, scalar1=0x8000, scalar2=0x3838,
                            op0=AT.bitwise_and, op1=AT.bitwise_or)
    nc.vector.tensor_tensor(out=dst[:], in0=tsA[:], in1=tsB[:],
                            op=AT.bitwise_or)


def _emit_program(nc, x_ap, res_ap, wt_ap, y_ap, b_ap, g_ap, be_ap,
                  scale_mul: float, use_b: bool, use_gamma: bool,
                  use_beta: bool):
    """Emit the per-core Tile program given DRAM APs.

    wt_ap is W TRANSPOSED ([INTER, HIDDEN]) -- a host-side layout choice so
    the weight lands k-major and needs no on-device transpose.
    """
    AT = mybir.AluOpType
    AF = mybir.ActivationFunctionType
    DRSI = mybir.MatmulPerfMode.DoubleRowSwInterleave
    K_TILES = INTER // 128            # 32 k-tiles of W^T
    WG = 4                            # k-tiles per W load
    from concourse.masks import make_identity

    with tile.TileContext(nc) as tc:
        with (
            tc.tile_pool(name="wt", bufs=1) as wt_pool,
            tc.tile_pool(name="wstage", bufs=3) as wstage_pool,
            tc.tile_pool(name="wpk", bufs=2) as wpk_pool,
            tc.tile_pool(name="const", bufs=1) as const_pool,
            tc.tile_pool(name="xio", bufs=3) as xio_pool,
            tc.tile_pool(name="xpk", bufs=2) as xpk_pool,
            tc.tile_pool(name="xt", bufs=3) as xt_pool,
            tc.tile_pool(name="res", bufs=4) as res_pool,
            tc.tile_pool(name="epi", bufs=2) as epi_pool,
            tc.tile_pool(name="stats", bufs=2) as stats_pool,
            tc.tile_pool(name="psum", bufs=2, space="PSUM") as psum_pool,
            tc.tile_pool(name="pst", bufs=2, space="PSUM") as pst_pool,
            tc.tile_pool(name="wsps", bufs=1, space="PSUM") as wsps_pool,
            tc.tile_pool(name="dram", bufs=1, space="DRAM") as dram_pool,
        ):
            epsT = const_pool.tile([128, 1], F32, tag="epsT")
            nc.vector.memset(epsT[:], float(EPS))
            ones1 = const_pool.tile([128, 1], BF16, tag="ones1")
            nc.vector.memset(ones1[:], 1.0)
            ident = const_pool.tile([128, 128], BF16, tag="ident")
            make_identity(nc, ident[:])

            # ---------------- DMA dispatch: W loads first ----------------
            # All big HBM loads go through the gpsimd (SWDGE) ring so they
            # drain in exactly this order: W^T groups, then x/res interleaved.
            wlds = []
            for g in range(K_TILES // WG):
                wld = wstage_pool.tile([128, WG, HIDDEN], BF16, tag="wld")
                nc.gpsimd.dma_start(
                    wld[:],
                    wt_ap[g * WG * 128:(g + 1) * WG * 128, :].rearrange(
                        "(c p) h -> p c h", p=128))
                wlds.append(wld)

            xins, inps = [], []
            for m in range(M_TILES):
                xin = xio_pool.tile([128, INTER], BF16, tag="xin")
                nc.gpsimd.dma_start(xin[:], x_ap[m * 128:(m + 1) * 128, :])
                xins.append(xin)
                inp = res_pool.tile([128, HIDDEN], F32, tag="inp")
                nc.gpsimd.dma_start(inp[:], res_ap[m * 128:(m + 1) * 128, :])
                inps.append(inp)

            # ---------------- W preparation (no transposes) ----------------
            # wT8 fp8 [128, 32, 1024]: (p, kt, h) = fp8 sign W[h, kt*128+p].
            # DoubleRow rhs block b, half n = [:, b::16, n*512:(n+1)*512]
            # (k-pair (b, b+16) matches the x pack pairing (c, 2048+c)).
            wT8 = wt_pool.tile([128, K_TILES, HIDDEN], FP8, tag="wT8",
                               name="wT8")
            wsps = wsps_pool.tile([1, HIDDEN], F32, tag="wsps", name="wsps")

            for kt in range(K_TILES):
                g, c = kt // WG, kt % WG
                wld = wlds[g]
                # fp8 sign directly on the scalar engine
                nc.scalar.sign(wT8[:, kt, :], wld[:, c, :])
                # |w| via sign-bit mask, then ones.T @ |w| accumulates the
                # per-channel scale numerator on the PE
                wabs = wpk_pool.tile([128, HIDDEN], BF16, tag="wabs")
                nc.vector.tensor_scalar(
                    out=wabs[:].bitcast(U16), in0=wld[:, c, :].bitcast(U16),
                    scalar1=0x7FFF, scalar2=None, op0=AT.bitwise_and)
                for n in range(2):
                    nc.tensor.matmul(wsps[:, n * 512:(n + 1) * 512],
                                     lhsT=ones1[:],
                                     rhs=wabs[:, n * 512:(n + 1) * 512],
                                     start=(kt == 0),
                                     stop=(kt == K_TILES - 1))

            # scale row -> DRAM -> broadcast to all partitions
            srow = const_pool.tile([1, HIDDEN], F32, tag="srow")
            nc.scalar.activation(srow[:], wsps[:], AF.Copy,
                                 scale=float(scale_mul))
            scratch = dram_pool.tile([HIDDEN], F32)
            nc.sync.dma_start(
                out=scratch[:].rearrange("(a n) -> a n", a=1), in_=srow[:])
            scaleF = const_pool.tile([128, HIDDEN], F32, tag="scaleF")
            nc.sync.dma_start(
                scaleF[:],
                scratch[:].rearrange("(a n) -> a n", a=1).broadcast_to([128, HIDDEN]))

            bB = gB = beB = None
            if use_b:
                bB = const_pool.tile([128, HIDDEN], F32, tag="bB")
                nc.sync.dma_start(
                    bB[:],
                    b_ap.rearrange("(a n) -> a n", a=1).broadcast_to([128, HIDDEN]))
            if use_gamma:
                gB = const_pool.tile([128, HIDDEN], F32, tag="gB")
                nc.sync.dma_start(
                    gB[:],
                    g_ap.rearrange("(a n) -> a n", a=1).broadcast_to([128, HIDDEN]))
            if use_beta:
                beB = const_pool.tile([128, HIDDEN], F32, tag="beB")
                nc.sync.dma_start(
                    beB[:],
                    be_ap.rearrange("(a n) -> a n", a=1).broadcast_to([128, HIDDEN]))

            # ---------------- main loop over token tiles ----------------
            # Software-pipelined: tile m's epilogue is emitted AFTER tile
            # m+1's pack/transpose/matmuls, so the in-order DVE/PE queues
            # never make the next tile's prep wait on the previous tile's
            # epilogue.
            def emit_front(m):
                xin = xins[m]
                xpackU = xpk_pool.tile([128, HALF], U16, tag="xpackU")
                _emit_pack(nc, xpk_pool, xin, xpackU, "xts")
                # transpose the 16 packed blocks on the PE (bit-exact for
                # the 4 sign-pair bf16 normals), staging through PSUM
                xTp = xt_pool.tile([128, NBLK, 128], U16, tag="xTp")
                for grp in range(2):
                    pst = pst_pool.tile([128, 8, 128], BF16, tag="pst")
                    for j in range(8):
                        blk = grp * 8 + j
                        nc.tensor.transpose(
                            pst[:, j, :],
                            xpackU[:, blk * 128:(blk + 1) * 128].bitcast(BF16),
                            ident[:])
                    nc.scalar.copy(
                        xTp[:, grp * 8:(grp + 1) * 8, :].bitcast(BF16),
                        pst[:])

                psum = psum_pool.tile([128, HIDDEN], F32, tag="psum",
                                      name="ps")
                for b in range(NBLK):
                    # forward interleaved byte-pairs; SwInterleave's column
                    # reversal is cancelled by the host-side row reversal
                    lhsT = xTp[:, b, :].bitcast(FP8)
                    for n in range(2):
                        nc.tensor.matmul(
                            psum[:, n * 512:(n + 1) * 512],
                            lhsT=lhsT,
                            rhs=wT8[:, b::NBLK, n * 512:(n + 1) * 512],
                            start=(b == 0), stop=(b == NBLK - 1),
                            perf_mode=DRSI)
                return psum

            def emit_epilogue(m, psum):
                # epilogue: r = psum * scaleF + inp (+ bB), then LayerNorm
                inp = inps[m]
                t = epi_pool.tile([128, HIDDEN], F32, tag="t")
                nc.vector.tensor_mul(t[:], psum[:], scaleF[:])
                r = epi_pool.tile([128, HIDDEN], F32, tag="r")
                nc.vector.tensor_add(r[:], t[:], inp[:])
                if use_b:
                    r2 = epi_pool.tile([128, HIDDEN], F32, tag="r2")
                    nc.vector.tensor_add(r2[:], r[:], bB[:])
                    r = r2

                bn6 = stats_pool.tile([128, 2, 6], F32, tag="bn6")
                nc.vector.bn_stats(bn6[:, 0, :], r[:, 0:512])
                nc.vector.bn_stats(bn6[:, 1, :], r[:, 512:1024])
                mv = stats_pool.tile([128, 2], F32, tag="mv")
                nc.vector.bn_aggr(mv[:], bn6[:])
                sd = stats_pool.tile([128, 1], F32, tag="sd")
                nc.scalar.activation(sd[:], mv[:, 1:2], AF.Sqrt,
                                     bias=epsT[:, 0:1])
                rstd = stats_pool.tile([128, 1], F32, tag="rstd")
                nc.vector.reciprocal(rstd[:], sd[:])
                nm = stats_pool.tile([128, 1], F32, tag="nm")
                nc.vector.tensor_scalar(out=nm[:], in0=mv[:, 0:1],
                                        scalar1=rstd[:, 0:1], scalar2=-1.0,
                                        op0=AT.mult, op1=AT.mult)
                y = epi_pool.tile([128, HIDDEN], F32, tag="y")
                nc.scalar.activation(y[:], r[:], AF.Identity,
                                     bias=nm[:, 0:1], scale=rstd[:, 0:1])
                if use_gamma:
                    y2 = epi_pool.tile([128, HIDDEN], F32, tag="y2")
                    nc.vector.tensor_mul(y2[:], y[:], gB[:])
                    y = y2
                if use_beta:
                    y3 = epi_pool.tile([128, HIDDEN], F32, tag="y3")
                    nc.vector.tensor_add(y3[:], y[:], beB[:])
                    y = y3

                nc.sync.dma_start(y_ap[m * 128:(m + 1) * 128, :], y[:])

            prev = None
            for m in range(M_TILES):
                psum = emit_front(m)
                if prev is not None:
                    emit_epilogue(m - 1, prev)
                prev = psum
            emit_epilogue(M_TILES - 1, prev)


def _build(scale_mul: float, use_b: bool, use_gamma: bool, use_beta: bool):
    """Build the SPMD program (identical on all 8 cores).

    scale_mul = |clip_val| / INTER, folded into the per-channel scale.
    """
    nc = bacc.Bacc("TRN2", target_bir_lowering=False, debug=False,
                   num_devices=N_CORES)

    x_ap = nc.dram_tensor("x", [TPC, INTER], F32, kind="ExternalInput").ap()
    res_ap = nc.dram_tensor("res", [TPC, HIDDEN], F32, kind="ExternalInput").ap()
    wt_ap = nc.dram_tensor("wt", [INTER, HIDDEN], F32, kind="ExternalInput").ap()
    b_ap = g_ap = be_ap = None
    if use_b:
        b_ap = nc.dram_tensor("bvec", [HIDDEN], F32, kind="ExternalInput").ap()
    if use_gamma:
        g_ap = nc.dram_tensor("gvec", [HIDDEN], F32, kind="ExternalInput").ap()
    if use_beta:
        be_ap = nc.dram_tensor("bevec", [HIDDEN], F32, kind="ExternalInput").ap()
    y_ap = nc.dram_tensor("y", [TPC, HIDDEN], F32, kind="ExternalOutput").ap()

    _emit_program(nc, x_ap, res_ap, wt_ap, y_ap, b_ap, g_ap, be_ap,
                  scale_mul, use_b, use_gamma, use_beta)
    nc.compile()
    return nc


_last_results = None


def kernel(hidden_states, input_tensor, W, b, clip_val, gamma, beta):
    global _last_results
    hidden_states = np.asarray(hidden_states)
    input_tensor = np.asarray(input_tensor)
    W = np.asarray(W, dtype=np.float32)
    b = np.asarray(b, dtype=np.float32)
    gamma = np.asarray(gamma, dtype=np.float32)
    beta = np.asarray(beta, dtype=np.float32)
    clip = float(np.asarray(clip_val))

    use_b = bool(np.any(b != 0.0))
    use_gamma = bool(np.any(gamma != 1.0))
    use_beta = bool(np.any(beta != 0.0))
    scale_mul = abs(clip) / INTER

    key = (scale_mul, use_b, use_gamma, use_beta)
    if key not in _cache:
        _cache[key] = _build(scale_mul, use_b, use_gamma, use_beta)
    nc = _cache[key]

    hs = np.ascontiguousarray(
        hidden_states.reshape(TOKENS, INTER).astype(np.float32, copy=False))
    rs = np.ascontiguousarray(
        input_tensor.reshape(TOKENS, HIDDEN).astype(np.float32, copy=False))
    Wc = np.ascontiguousarray(W.T)   # layout choice: weight fed k-major

    in_maps = []
    for c in range(N_CORES):
        m = {
            "x": _prepare_x(hs[c * TPC:(c + 1) * TPC]),
            "res": np.ascontiguousarray(rs[c * TPC:(c + 1) * TPC]),
            "wt": Wc,
        }
        if use_b:
            m["bvec"] = b
        if use_gamma:
            m["gvec"] = gamma
        if use_beta:
            m["bevec"] = beta
        in_maps.append(m)

    kwargs = {}
    if TRACE:
        _install_ntff_hook()
        kwargs["trace"] = True
        if TRACE_ALL_CORES:
            kwargs["trace_cores"] = list(range(N_CORES))
    res = bass_utils.run_bass_kernel_spmd(
        nc, in_maps, core_ids=list(range(N_CORES)), **kwargs)
    _last_results = res

    y = np.concatenate([res.results[c]["y"] for c in range(N_CORES)], axis=0)
    return y.reshape(hidden_states.shape[:-1] + (HIDDEN,)).astype(np.float32)
